# revision 26
# baseline (speedup 1.0000x reference)
"""DEMA (double exponential smoothing) Trainium2 kernel.

x: [64, 2048, 512] fp32; recurrence over T=2048 is a 2x2 linear
time-invariant system per (batch, channel) lane:

    z_t = A z_{t-1} + B x_t,   y_t = e1^T z_t
    A = [[1-a, 1-a], [-ab, 1-ab]],  B = [a, ab]^T

Blocked scan: chunks of L=126 timesteps. One [128x128] @ [128x512]
matmul per (batch, chunk): rhs rows 0-1 carry the (s, b) state into
the chunk, rows 2..127 carry the chunk's inputs; lhsT columns 0-1
produce the chunk-end state (fed into the next chunk's rhs rows 0-1
via a tiny PSUM->SBUF copy), columns 2..127 produce the outputs.

The kernel is HBM-bandwidth bound (in+out traffic), so everything on
the wire is bf16: the host quantizes x and the chunk matrices to bf16
(tolerance is 2e-2 relative; bf16 end-to-end lands ~5e-3) and the
device writes bf16 outputs that the host upcasts. This halves HBM
traffic vs fp32 and runs the PE at 1 cycle/row instead of fp32's 4.
Batches are sharded 8 ways across cores; within a core the 8 batches
are packed as 4 channel-interleaved pairs so every DMA row is a 2KB
descriptor, and the 4 pair-chains are interleaved chunk-major so the
PE always has independent work while each carry chain advances.
"""

import sys

import numpy as np

if "/opt/trn_rl_repo" not in sys.path:
    sys.path.insert(0, "/opt/trn_rl_repo")

B, T, C = 64, 2048, 512
NCORES = 8
BPC = B // NCORES  # batches per core
GB = 2             # batches packed side-by-side per tile (2KB DMA rows)
NP = BPC // GB     # batch-pair chains per core
W = GB * C         # tile width in the free dim
L = 126            # timesteps per full chunk (126 outputs + 2 state rows = 128)
NFULL = 16         # full chunks cover t = 0..2015
LT = T - NFULL * L  # tail chunk, 32 timesteps

_cache = {}


def _bf16():
    from ml_dtypes import bfloat16

    return bfloat16


def _build_mats(alpha, beta):
    """Per-call host precompute of the chunk transfer matrices (float64)."""
    a = np.float64(alpha)
    b = np.float64(beta)
    A = np.array([[1 - a, 1 - a], [-a * b, 1 - a * b]], dtype=np.float64)
    Bv = np.array([a, a * b], dtype=np.float64)
    NPOW = L + LT + 1
    Ap = [np.eye(2)]
    for _ in range(NPOW):
        Ap.append(Ap[-1] @ A)
    AB = np.stack([Ap[j] @ Bv for j in range(NPOW)])  # [NPOW, 2], A^j B
    w = AB[:, 0]                                      # w_j = e1^T A^j B

    # Generic chunk starting at t0, carry z_{t0-1} in rhs rows 0-1:
    #   z_{t0+tau} = A^{tau+1} z_{t0-1} + sum_k A^{tau-k} B x_{t0+k}
    G1 = np.zeros((128, 128))
    for tau in range(L):
        m = 2 + tau
        G1[0, m] = Ap[tau + 1][0, 0]
        G1[1, m] = Ap[tau + 1][0, 1]
        for k in range(tau + 1):
            G1[2 + k, m] = w[tau - k]
    for j in range(2):
        for jp in range(2):
            G1[j, jp] = Ap[L][jp, j]
    for k in range(L):
        G1[2 + k, 0] = AB[L - 1 - k][0]
        G1[2 + k, 1] = AB[L - 1 - k][1]

    # Chunk 0: z_0 = (x_0, x_1 - x_0), y_0 = x_0, rhs rows 0-1 are zero.
    G0 = np.zeros((128, 128))
    G0[2, 2] = 1.0
    for tau in range(1, L):
        m = 2 + tau
        G0[2, m] = Ap[tau][0, 0] - Ap[tau][0, 1]
        G0[3, m] = Ap[tau][0, 1] + w[tau - 1]
        for k in range(2, tau + 1):
            G0[2 + k, m] = w[tau - k]
    for jp in range(2):
        G0[2, jp] = Ap[L - 1][jp, 0] - Ap[L - 1][jp, 1]
        G0[3, jp] = Ap[L - 1][jp, 1] + AB[L - 2][jp]
        for k in range(2, L):
            G0[2 + k, jp] = AB[L - 1 - k][jp]

    # Exact tail: y_{t0+L+tau} expanded directly over chunk 15's rhs
    # (carry z_{t0-1} at rows 0-1, inputs x_{t0..t0+L-1} at rows 2..127)
    # plus the tail's own inputs — no carry copy between chunk 15 and
    # the tail, so the end-game has no vector-engine dependency.
    Gt2 = np.zeros((128, LT))
    for tau in range(LT):
        Gt2[0, tau] = Ap[tau + 1 + L][0, 0]
        Gt2[1, tau] = Ap[tau + 1 + L][0, 1]
        for k in range(L):
            Gt2[2 + k, tau] = w[tau + L - k]
    Gt3 = np.zeros((LT, LT))
    for tau in range(LT):
        for k in range(tau + 1):
            Gt3[k, tau] = w[tau - k]
    bf16 = _bf16()
    # drop G0's all-zero carry rows: round 0's rhs has inputs at rows 0..L-1
    return (
        G0[2:128].astype(bf16),
        G1.astype(bf16),
        Gt2.astype(bf16),
        Gt3.astype(bf16),
    )


def _build_program():
    import concourse.mybir as mybir
    import concourse.tile as tile
    from concourse import bacc

    FP32 = mybir.dt.float32
    BF16 = mybir.dt.bfloat16
    nc = bacc.Bacc(
        "TRN2", target_bir_lowering=False, debug=False, enable_asserts=False
    )
    x_d = nc.dram_tensor("x", [NP, T, W], BF16, kind="ExternalInput").ap()
    # g0 is the chunk-0 matrix with the (all-zero) carry rows dropped:
    # [126, 128], so round 0's rhs needs no zeroed carry rows.
    g0_d = nc.dram_tensor("g0", [L, 128], BF16, kind="ExternalInput").ap()
    g1_d = nc.dram_tensor("g1", [128, 128], BF16, kind="ExternalInput").ap()
    gt2_d = nc.dram_tensor("gt2", [128, LT], BF16, kind="ExternalInput").ap()
    gt3_d = nc.dram_tensor("gt3", [LT, LT], BF16, kind="ExternalInput").ap()
    y_d = nc.dram_tensor("y", [NP, T, W], BF16, kind="ExternalOutput").ap()

    with tile.TileContext(nc) as tc:
        with (
            tc.tile_pool(name="g", bufs=1) as gpool,
            tc.tile_pool(name="xp", bufs=12) as xpool,
            tc.tile_pool(name="op", bufs=10) as opool,
            tc.tile_pool(name="ps", bufs=4, space="PSUM") as pspool,
        ):
            g0 = gpool.tile([L, 128], BF16, tag="g0")
            g1 = gpool.tile([128, 128], BF16, tag="g1")
            gt2 = gpool.tile([128, LT], BF16, tag="gt2")
            gt3 = gpool.tile([LT, LT], BF16, tag="gt3")
            # issue order tuned for the startup ramp: the first matmul
            # needs only g0 + chain-0's tile, so those go first on sync;
            # scalar's queue starts behind the auto-inserted ACT table
            # load, so it gets the later-needed tiles and g1/gt
            nc.sync.dma_start(out=g0[:], in_=g0_d)

            xcur = []
            for p in range(NP):
                xs = xpool.tile([L, W], BF16, tag="x")
                eng = nc.sync if p < 2 else nc.scalar
                eng.dma_start(out=xs[:], in_=x_d[p, 0:L, :])
                xcur.append(xs)
            nc.scalar.dma_start(out=g1[:], in_=g1_d)
            nc.scalar.dma_start(out=gt2[:], in_=gt2_d)
            nc.scalar.dma_start(out=gt3[:], in_=gt3_d)

            xprev = [None] * NP
            for i in range(NFULL):
                for p in range(NP):
                    xs = xcur[p]
                    ps = pspool.tile([128, W], FP32, tag="ps")
                    # next round's input tile for this chain; issue the
                    # prefetch DMA before this round's matmuls so the
                    # DMA queues never starve. Round 0's prefetches go
                    # through gpsimd's SWDGE so three queues ramp the
                    # DMA engines in parallel at startup.
                    if i + 1 < NFULL:
                        nxt = xpool.tile([128, W], BF16, tag="x")
                        nrows, ofs = L, 2
                    else:
                        # tail tile: pure inputs, no carry rows (the tail
                        # is computed from chunk 15's rhs + these rows)
                        nxt = xpool.tile([LT, W], BF16, tag="x")
                        nrows, ofs = LT, 0
                    peng = nc.gpsimd if i == 0 else nc.sync
                    peng.dma_start(
                        out=nxt[ofs:ofs + nrows, :],
                        in_=x_d[p, L * (i + 1):L * (i + 1) + nrows, :],
                    )
                    for bb in range(GB):
                        sl = slice(bb * C, (bb + 1) * C)
                        if i == 0:
                            nc.tensor.matmul(
                                ps[:, sl], g0[:], xs[0:L, sl],
                                start=True, stop=True,
                            )
                        else:
                            nc.tensor.matmul(
                                ps[:, sl], g1[:], xs[:, sl],
                                start=True, stop=True,
                            )
                    if i < NFULL - 1:
                        # chunk-end states -> next rhs rows 0-1 (fp32->bf16)
                        nc.vector.tensor_copy(out=nxt[0:2, :], in_=ps[0:2, :])
                    else:
                        xprev[p] = xs  # keep chunk 15's rhs for the tail
                    xcur[p] = nxt
                    o = opool.tile([128, W], BF16, tag="o")
                    # at the end the vector engine has no carries left, so
                    # chunk 15's drains split scalar/vector to halve the
                    # serial drain chain in front of the final outputs
                    if i == NFULL - 1 and p % 2 == 1:
                        nc.vector.tensor_copy(out=o[:], in_=ps[:])
                    else:
                        nc.scalar.copy(out=o[:], in_=ps[:])
                    # keep the last rounds' outputs off gpsimd, whose SWDGE
                    # end-of-kernel DRAIN takes ~5us after the final
                    # software-DGE DMA completes, while the HWDGE queues
                    # drain in ~200ns
                    eng = nc.sync if i == NFULL - 1 else nc.gpsimd
                    eng.dma_start(
                        out=y_d[p, L * i:L * (i + 1), :],
                        in_=o[2:128, :],
                    )

            # exact tail (32 steps): two accumulating matmuls per slice —
            # chunk 15's rhs through Gt2 plus the tail inputs through Gt3
            for p in range(NP):
                ps2 = pspool.tile([128, W], FP32, tag="ps")
                for bb in range(GB):
                    sl = slice(bb * C, (bb + 1) * C)
                    nc.tensor.matmul(
                        ps2[0:LT, sl], gt2[:], xprev[p][:, sl],
                        start=True, stop=False,
                    )
                    nc.tensor.matmul(
                        ps2[0:LT, sl], gt3[:], xcur[p][0:LT, sl],
                        start=False, stop=True,
                    )
                o2 = opool.tile([LT, W], BF16, tag="o")
                if p % 2 == 1:
                    nc.vector.tensor_copy(out=o2[:], in_=ps2[0:LT, :])
                else:
                    nc.scalar.copy(out=o2[:], in_=ps2[0:LT, :])
                nc.sync.dma_start(
                    out=y_d[p, L * NFULL:T, :],
                    in_=o2[:],
                )
    nc.compile()
    return nc


def _get_program():
    if "nc" not in _cache:
        _cache["nc"] = _build_program()
    return _cache["nc"]


def _ensure_axon_hooks_shim():
    """concourse's trace path does `from antenv.axon_hooks import ...`;
    some images lack that module. Install a no-op shim so an externally
    set BASS_TRACE can't crash the run (tracing then degrades to off)."""
    import types

    try:
        import antenv.axon_hooks  # noqa: F401
        return
    except ImportError:
        pass
    try:
        import antenv
    except ImportError:
        return
    mod = types.ModuleType("antenv.axon_hooks")
    mod.get_axon_ntff_profile_hook = lambda: None
    mod.set_axon_ntff_profile_hook = lambda h: None
    mod._kernel_shim = True
    sys.modules["antenv.axon_hooks"] = mod
    antenv.axon_hooks = mod


def _run(x, alpha, beta, trace=False):
    _ensure_axon_hooks_shim()
    from concourse.bass_utils import run_bass_kernel_spmd

    bf16 = _bf16()
    x = np.asarray(x, dtype=np.float32)
    # bf16-quantize, then interleave batch pairs channel-wise:
    # [64, T, C] -> [32, T, 2C] with batches (2p, 2p+1) side by side
    xb = (
        x.astype(bf16)
        .reshape(B // GB, GB, T, C)
        .transpose(0, 2, 1, 3)
        .reshape(B // GB, T, W)
    )
    G0, G1, Gt2, Gt3 = _build_mats(alpha, beta)
    nc = _get_program()
    in_maps = [
        {"x": xb[c * NP:(c + 1) * NP], "g0": G0, "g1": G1,
         "gt2": Gt2, "gt3": Gt3}
        for c in range(NCORES)
    ]
    res = run_bass_kernel_spmd(nc, in_maps, list(range(NCORES)), trace=trace)
    yb = np.concatenate([res.results[c]["y"] for c in range(NCORES)], axis=0)
    out = (
        yb.reshape(B // GB, T, GB, C)
        .transpose(0, 2, 1, 3)
        .astype(np.float32)
        .reshape(B, T, C)
    )
    return out, res


def kernel(**inputs):
    alpha = float(np.asarray(inputs["alpha"]))
    beta = float(np.asarray(inputs["beta"]))
    out, _ = _run(inputs["x"], alpha, beta, trace=False)
    return out


# revision 29
# speedup vs baseline: 1.0504x; 1.0504x over previous
"""DEMA (double exponential smoothing) Trainium2 kernel.

x: [64, 2048, 512] fp32; recurrence over T=2048 is a 2x2 linear
time-invariant system per (batch, channel) lane:

    z_t = A z_{t-1} + B x_t,   y_t = e1^T z_t
    A = [[1-a, 1-a], [-ab, 1-ab]],  B = [a, ab]^T

Blocked scan: chunks of L=126 timesteps. One [128x128] @ [128x512]
matmul per (batch, chunk): rhs rows 0-1 carry the (s, b) state into
the chunk, rows 2..127 carry the chunk's inputs; lhsT columns 0-1
produce the chunk-end state (fed into the next chunk's rhs rows 0-1
via a tiny PSUM->SBUF copy), columns 2..127 produce the outputs.

The kernel is HBM-bandwidth bound (in+out traffic), so everything on
the wire is bf16: the host quantizes x and the chunk matrices to bf16
(tolerance is 2e-2 relative; bf16 end-to-end lands ~5e-3) and the
device writes bf16 outputs that the host upcasts. This halves HBM
traffic vs fp32 and runs the PE at 1 cycle/row instead of fp32's 4.
Batches are sharded 8 ways across cores; within a core the 8 batches
are packed as 4 channel-interleaved pairs so every DMA row is a 2KB
descriptor, and the 4 pair-chains are interleaved chunk-major so the
PE always has independent work while each carry chain advances.
"""

import sys

import numpy as np

if "/opt/trn_rl_repo" not in sys.path:
    sys.path.insert(0, "/opt/trn_rl_repo")

B, T, C = 64, 2048, 512
NCORES = 8
BPC = B // NCORES  # batches per core
GB = 2             # batches packed side-by-side per tile (2KB DMA rows)
NP = BPC // GB     # batch-pair chains per core
W = GB * C         # tile width in the free dim
L = 126            # timesteps per full chunk (126 outputs + 2 state rows = 128)
NFULL = 16         # full chunks cover t = 0..2015
LT = T - NFULL * L  # tail chunk, 32 timesteps

_cache = {}


def _bf16():
    from ml_dtypes import bfloat16

    return bfloat16


def _build_mats(alpha, beta):
    """Per-call host precompute of the chunk transfer matrices (float64)."""
    a = np.float64(alpha)
    b = np.float64(beta)
    A = np.array([[1 - a, 1 - a], [-a * b, 1 - a * b]], dtype=np.float64)
    Bv = np.array([a, a * b], dtype=np.float64)
    Ap = [np.eye(2)]
    for _ in range(L):
        Ap.append(Ap[-1] @ A)
    AB = np.stack([Ap[j] @ Bv for j in range(L)])  # [L, 2], A^j B
    w = AB[:, 0]                                   # w_j = e1^T A^j B

    # Generic chunk starting at t0, carry z_{t0-1} in rhs rows 0-1:
    #   z_{t0+tau} = A^{tau+1} z_{t0-1} + sum_k A^{tau-k} B x_{t0+k}
    G1 = np.zeros((128, 128))
    for tau in range(L):
        m = 2 + tau
        G1[0, m] = Ap[tau + 1][0, 0]
        G1[1, m] = Ap[tau + 1][0, 1]
        for k in range(tau + 1):
            G1[2 + k, m] = w[tau - k]
    for j in range(2):
        for jp in range(2):
            G1[j, jp] = Ap[L][jp, j]
    for k in range(L):
        G1[2 + k, 0] = AB[L - 1 - k][0]
        G1[2 + k, 1] = AB[L - 1 - k][1]

    # Chunk 0: z_0 = (x_0, x_1 - x_0), y_0 = x_0, rhs rows 0-1 are zero.
    G0 = np.zeros((128, 128))
    G0[2, 2] = 1.0
    for tau in range(1, L):
        m = 2 + tau
        G0[2, m] = Ap[tau][0, 0] - Ap[tau][0, 1]
        G0[3, m] = Ap[tau][0, 1] + w[tau - 1]
        for k in range(2, tau + 1):
            G0[2 + k, m] = w[tau - k]
    for jp in range(2):
        G0[2, jp] = Ap[L - 1][jp, 0] - Ap[L - 1][jp, 1]
        G0[3, jp] = Ap[L - 1][jp, 1] + AB[L - 2][jp]
        for k in range(2, L):
            G0[2 + k, jp] = AB[L - 1 - k][jp]

    # Tail chunk: LT outputs, no state columns.
    Gt = np.zeros((2 + LT, LT))
    for tau in range(LT):
        Gt[0, tau] = Ap[tau + 1][0, 0]
        Gt[1, tau] = Ap[tau + 1][0, 1]
        for k in range(tau + 1):
            Gt[2 + k, tau] = w[tau - k]
    bf16 = _bf16()
    # drop G0's all-zero carry rows: round 0's rhs has inputs at rows 0..L-1
    return (
        G0[2:128].astype(bf16),
        G1.astype(bf16),
        Gt.astype(bf16),
    )


def _build_program():
    import concourse.mybir as mybir
    import concourse.tile as tile
    from concourse import bacc

    FP32 = mybir.dt.float32
    BF16 = mybir.dt.bfloat16
    nc = bacc.Bacc(
        "TRN2", target_bir_lowering=False, debug=False, enable_asserts=False
    )
    x_d = nc.dram_tensor("x", [NP, T, W], BF16, kind="ExternalInput").ap()
    # g0 is the chunk-0 matrix with the (all-zero) carry rows dropped:
    # [126, 128], so round 0's rhs needs no zeroed carry rows.
    g0_d = nc.dram_tensor("g0", [L, 128], BF16, kind="ExternalInput").ap()
    g1_d = nc.dram_tensor("g1", [128, 128], BF16, kind="ExternalInput").ap()
    gt_d = nc.dram_tensor("gt", [2 + LT, LT], BF16, kind="ExternalInput").ap()
    y_d = nc.dram_tensor("y", [NP, T, W], BF16, kind="ExternalOutput").ap()

    with tile.TileContext(nc) as tc:
        with (
            tc.tile_pool(name="g", bufs=1) as gpool,
            tc.tile_pool(name="xp", bufs=12) as xpool,
            tc.tile_pool(name="op", bufs=10) as opool,
            tc.tile_pool(name="ps", bufs=4, space="PSUM") as pspool,
        ):
            g0 = gpool.tile([L, 128], BF16, tag="g0")
            g1 = gpool.tile([128, 128], BF16, tag="g1")
            gt = gpool.tile([2 + LT, LT], BF16, tag="gt")
            # issue order tuned for the startup ramp: the first matmul
            # needs only g0 + chain-0's tile, so those go first on sync;
            # scalar's queue starts behind the auto-inserted ACT table
            # load, so it gets the later-needed tiles and g1/gt
            nc.sync.dma_start(out=g0[:], in_=g0_d)

            xcur = []
            for p in range(NP):
                xs = xpool.tile([L, W], BF16, tag="x")
                eng = nc.sync if p < 2 else nc.scalar
                eng.dma_start(out=xs[:], in_=x_d[p, 0:L, :])
                xcur.append(xs)
            nc.scalar.dma_start(out=g1[:], in_=g1_d)
            nc.scalar.dma_start(out=gt[:], in_=gt_d)

            for i in range(NFULL + 1):
                for p in range(NP):
                    xs = xcur[p]
                    ps = pspool.tile([128, W], FP32, tag="ps")
                    if i < NFULL:
                        # next round's input tile for this chain; issue the
                        # prefetch DMA before this round's matmuls so the
                        # DMA queues never starve
                        if i + 1 < NFULL:
                            nxt = xpool.tile([128, W], BF16, tag="x")
                            nrows = L
                        else:
                            nxt = xpool.tile([2 + LT, W], BF16, tag="x")
                            nrows = LT
                        peng = (nc.sync if (i > 0 or p < 2) else nc.scalar)
                        peng.dma_start(
                            out=nxt[2:2 + nrows, :],
                            in_=x_d[p, L * (i + 1):L * (i + 1) + nrows, :],
                        )
                        for bb in range(GB):
                            sl = slice(bb * C, (bb + 1) * C)
                            if i == 0:
                                nc.tensor.matmul(
                                    ps[:, sl], g0[:], xs[0:L, sl],
                                    start=True, stop=True,
                                )
                            else:
                                nc.tensor.matmul(
                                    ps[:, sl], g1[:], xs[:, sl],
                                    start=True, stop=True,
                                )
                        # chunk-end states -> next rhs rows 0-1 (fp32->bf16)
                        nc.vector.tensor_copy(out=nxt[0:2, :], in_=ps[0:2, :])
                        xcur[p] = nxt
                        o = opool.tile([128, W], BF16, tag="o")
                        nc.scalar.copy(out=o[:], in_=ps[:])
                        # keep the last two rounds' outputs off gpsimd:
                        # its SWDGE end-of-kernel DRAIN takes ~5us after
                        # the final software-DGE DMA completes, while the
                        # HWDGE queues drain in ~200ns
                        if i >= NFULL - 1:
                            eng = (nc.sync, nc.scalar)[p % 2]
                        else:
                            eng = nc.gpsimd
                        eng.dma_start(
                            out=y_d[p, L * i:L * (i + 1), :],
                            in_=o[2:128, :],
                        )
                    else:  # tail chunk (32 steps, no state outputs)
                        for bb in range(GB):
                            sl = slice(bb * C, (bb + 1) * C)
                            nc.tensor.matmul(
                                ps[0:LT, sl], gt[:], xs[0:2 + LT, sl],
                                start=True, stop=True,
                            )
                        o = opool.tile([LT, W], BF16, tag="o")
                        nc.scalar.copy(out=o[:], in_=ps[0:LT, :])
                        eng = (nc.scalar, nc.sync)[p % 2]
                        eng.dma_start(
                            out=y_d[p, L * NFULL:T, :],
                            in_=o[:],
                        )
    nc.compile()
    return nc


def _get_program():
    if "nc" not in _cache:
        _cache["nc"] = _build_program()
    return _cache["nc"]


def _ensure_axon_hooks_shim():
    """concourse's trace path does `from antenv.axon_hooks import ...`;
    some images lack that module. Install a no-op shim so an externally
    set BASS_TRACE can't crash the run (tracing then degrades to off)."""
    import types

    try:
        import antenv.axon_hooks  # noqa: F401
        return
    except ImportError:
        pass
    try:
        import antenv
    except ImportError:
        return
    mod = types.ModuleType("antenv.axon_hooks")
    mod.get_axon_ntff_profile_hook = lambda: None
    mod.set_axon_ntff_profile_hook = lambda h: None
    mod._kernel_shim = True
    sys.modules["antenv.axon_hooks"] = mod
    antenv.axon_hooks = mod


def _run(x, alpha, beta, trace=False):
    _ensure_axon_hooks_shim()
    from concourse.bass_utils import run_bass_kernel_spmd

    bf16 = _bf16()
    x = np.asarray(x, dtype=np.float32)
    # bf16-quantize, then interleave batch pairs channel-wise:
    # [64, T, C] -> [32, T, 2C] with batches (2p, 2p+1) side by side
    xb = (
        x.astype(bf16)
        .reshape(B // GB, GB, T, C)
        .transpose(0, 2, 1, 3)
        .reshape(B // GB, T, W)
    )
    G0, G1, Gt = _build_mats(alpha, beta)
    nc = _get_program()
    in_maps = [
        {"x": xb[c * NP:(c + 1) * NP], "g0": G0, "g1": G1, "gt": Gt}
        for c in range(NCORES)
    ]
    res = run_bass_kernel_spmd(nc, in_maps, list(range(NCORES)), trace=trace)
    yb = np.concatenate([res.results[c]["y"] for c in range(NCORES)], axis=0)
    out = (
        yb.reshape(B // GB, T, GB, C)
        .transpose(0, 2, 1, 3)
        .astype(np.float32)
        .reshape(B, T, C)
    )
    return out, res


def kernel(**inputs):
    alpha = float(np.asarray(inputs["alpha"]))
    beta = float(np.asarray(inputs["beta"]))
    out, _ = _run(inputs["x"], alpha, beta, trace=False)
    return out


# revision 31
# speedup vs baseline: 1.0673x; 1.0161x over previous
"""DEMA (double exponential smoothing) Trainium2 kernel.

x: [64, 2048, 512] fp32; recurrence over T=2048 is a 2x2 linear
time-invariant system per (batch, channel) lane:

    z_t = A z_{t-1} + B x_t,   y_t = e1^T z_t
    A = [[1-a, 1-a], [-ab, 1-ab]],  B = [a, ab]^T

Blocked scan: chunks of L=126 timesteps. One [128x128] @ [128x512]
matmul per (batch, chunk): rhs rows 0-1 carry the (s, b) state into
the chunk, rows 2..127 carry the chunk's inputs; lhsT columns 0-1
produce the chunk-end state (fed into the next chunk's rhs rows 0-1
via a tiny PSUM->SBUF copy), columns 2..127 produce the outputs.

The kernel is HBM-bandwidth bound (in+out traffic), so everything on
the wire is bf16: the host quantizes x and the chunk matrices to bf16
(tolerance is 2e-2 relative; bf16 end-to-end lands ~5e-3) and the
device writes bf16 outputs that the host upcasts. This halves HBM
traffic vs fp32 and runs the PE at 1 cycle/row instead of fp32's 4.
Batches are sharded 8 ways across cores; within a core the 8 batches
are packed as 4 channel-interleaved pairs so every DMA row is a 2KB
descriptor, and the 4 pair-chains are interleaved chunk-major so the
PE always has independent work while each carry chain advances.
"""

import sys

import numpy as np

if "/opt/trn_rl_repo" not in sys.path:
    sys.path.insert(0, "/opt/trn_rl_repo")

B, T, C = 64, 2048, 512
NCORES = 8
BPC = B // NCORES  # batches per core
GB = 2             # batches packed side-by-side per tile (2KB DMA rows)
NP = BPC // GB     # batch-pair chains per core
W = GB * C         # tile width in the free dim
L = 126            # timesteps per full chunk (126 outputs + 2 state rows = 128)
NFULL = 16         # full chunks cover t = 0..2015
LT = T - NFULL * L  # tail chunk, 32 timesteps

_cache = {}


def _bf16():
    from ml_dtypes import bfloat16

    return bfloat16


def _build_mats(alpha, beta):
    """Per-call host precompute of the chunk transfer matrices (float64)."""
    a = np.float64(alpha)
    b = np.float64(beta)
    A = np.array([[1 - a, 1 - a], [-a * b, 1 - a * b]], dtype=np.float64)
    Bv = np.array([a, a * b], dtype=np.float64)
    Ap = [np.eye(2)]
    for _ in range(L):
        Ap.append(Ap[-1] @ A)
    AB = np.stack([Ap[j] @ Bv for j in range(L)])  # [L, 2], A^j B
    w = AB[:, 0]                                   # w_j = e1^T A^j B

    # Generic chunk starting at t0, carry z_{t0-1} in rhs rows 0-1:
    #   z_{t0+tau} = A^{tau+1} z_{t0-1} + sum_k A^{tau-k} B x_{t0+k}
    G1 = np.zeros((128, 128))
    for tau in range(L):
        m = 2 + tau
        G1[0, m] = Ap[tau + 1][0, 0]
        G1[1, m] = Ap[tau + 1][0, 1]
        for k in range(tau + 1):
            G1[2 + k, m] = w[tau - k]
    for j in range(2):
        for jp in range(2):
            G1[j, jp] = Ap[L][jp, j]
    for k in range(L):
        G1[2 + k, 0] = AB[L - 1 - k][0]
        G1[2 + k, 1] = AB[L - 1 - k][1]

    # Chunk 0: z_0 = (x_0, x_1 - x_0), y_0 = x_0, rhs rows 0-1 are zero.
    G0 = np.zeros((128, 128))
    G0[2, 2] = 1.0
    for tau in range(1, L):
        m = 2 + tau
        G0[2, m] = Ap[tau][0, 0] - Ap[tau][0, 1]
        G0[3, m] = Ap[tau][0, 1] + w[tau - 1]
        for k in range(2, tau + 1):
            G0[2 + k, m] = w[tau - k]
    for jp in range(2):
        G0[2, jp] = Ap[L - 1][jp, 0] - Ap[L - 1][jp, 1]
        G0[3, jp] = Ap[L - 1][jp, 1] + AB[L - 2][jp]
        for k in range(2, L):
            G0[2 + k, jp] = AB[L - 1 - k][jp]

    # Tail chunk: LT outputs, no state columns.
    Gt = np.zeros((2 + LT, LT))
    for tau in range(LT):
        Gt[0, tau] = Ap[tau + 1][0, 0]
        Gt[1, tau] = Ap[tau + 1][0, 1]
        for k in range(tau + 1):
            Gt[2 + k, tau] = w[tau - k]
    bf16 = _bf16()
    # drop G0's all-zero carry rows: round 0's rhs has inputs at rows 0..L-1
    return (
        G0[2:128].astype(bf16),
        G1.astype(bf16),
        Gt.astype(bf16),
    )


def _build_program():
    import concourse.mybir as mybir
    import concourse.tile as tile
    from concourse import bacc

    FP32 = mybir.dt.float32
    BF16 = mybir.dt.bfloat16
    nc = bacc.Bacc(
        "TRN2", target_bir_lowering=False, debug=False, enable_asserts=False
    )
    x_d = nc.dram_tensor("x", [NP, T, W], BF16, kind="ExternalInput").ap()
    # g0 is the chunk-0 matrix with the (all-zero) carry rows dropped:
    # [126, 128], so round 0's rhs needs no zeroed carry rows.
    g0_d = nc.dram_tensor("g0", [L, 128], BF16, kind="ExternalInput").ap()
    g1_d = nc.dram_tensor("g1", [128, 128], BF16, kind="ExternalInput").ap()
    gt_d = nc.dram_tensor("gt", [2 + LT, LT], BF16, kind="ExternalInput").ap()
    y_d = nc.dram_tensor("y", [NP, T, W], BF16, kind="ExternalOutput").ap()

    with tile.TileContext(nc) as tc:
        with (
            tc.tile_pool(name="g", bufs=1) as gpool,
            tc.tile_pool(name="xp", bufs=12) as xpool,
            tc.tile_pool(name="op", bufs=10) as opool,
            tc.tile_pool(name="ps", bufs=4, space="PSUM") as pspool,
        ):
            g0 = gpool.tile([L, 128], BF16, tag="g0")
            g1 = gpool.tile([128, 128], BF16, tag="g1")
            gt = gpool.tile([2 + LT, LT], BF16, tag="gt")
            # issue order tuned for the startup ramp: the first matmul
            # needs only g0 + chain-0's tile, so those go first on sync;
            # scalar's queue starts behind the auto-inserted ACT table
            # load, so it gets the later-needed tiles and g1/gt
            nc.sync.dma_start(out=g0[:], in_=g0_d)

            xcur = []
            for p in range(NP):
                xs = xpool.tile([L, W], BF16, tag="x")
                eng = nc.sync if p < 2 else nc.scalar
                eng.dma_start(out=xs[:], in_=x_d[p, 0:L, :])
                xcur.append(xs)
            nc.scalar.dma_start(out=g1[:], in_=g1_d)
            nc.scalar.dma_start(out=gt[:], in_=gt_d)

            for i in range(NFULL + 1):
                for p in range(NP):
                    xs = xcur[p]
                    ps = pspool.tile([128, W], FP32, tag="ps")
                    if i < NFULL:
                        # next round's input tile for this chain; issue the
                        # prefetch DMA before this round's matmuls so the
                        # DMA queues never starve
                        if i + 1 < NFULL:
                            nxt = xpool.tile([128, W], BF16, tag="x")
                            nrows = L
                        else:
                            nxt = xpool.tile([2 + LT, W], BF16, tag="x")
                            nrows = LT
                        peng = (nc.sync if (i > 0 or p < 2) else nc.scalar)
                        peng.dma_start(
                            out=nxt[2:2 + nrows, :],
                            in_=x_d[p, L * (i + 1):L * (i + 1) + nrows, :],
                        )
                        for bb in range(GB):
                            sl = slice(bb * C, (bb + 1) * C)
                            if i == 0:
                                nc.tensor.matmul(
                                    ps[:, sl], g0[:], xs[0:L, sl],
                                    start=True, stop=True,
                                )
                            else:
                                nc.tensor.matmul(
                                    ps[:, sl], g1[:], xs[:, sl],
                                    start=True, stop=True,
                                )
                        # chunk-end states -> next rhs rows 0-1 (fp32->bf16)
                        nc.vector.tensor_copy(out=nxt[0:2, :], in_=ps[0:2, :])
                        xcur[p] = nxt
                        o = opool.tile([128, W], BF16, tag="o")
                        nc.scalar.copy(out=o[:], in_=ps[:])
                        # keep the last two rounds' outputs off gpsimd:
                        # its SWDGE end-of-kernel DRAIN takes ~5us after
                        # the final software-DGE DMA completes, while the
                        # HWDGE queues drain in ~200ns
                        # ... and keep the end output ISSUES off scalar too:
                        # its serial queue of 8 final drains plus 4 issues
                        # is exactly the measured 11.7us end-game
                        if i >= NFULL - 1:
                            eng = nc.sync
                        else:
                            eng = nc.gpsimd
                        eng.dma_start(
                            out=y_d[p, L * i:L * (i + 1), :],
                            in_=o[2:128, :],
                        )
                    else:  # tail chunk (32 steps, no state outputs)
                        for bb in range(GB):
                            sl = slice(bb * C, (bb + 1) * C)
                            nc.tensor.matmul(
                                ps[0:LT, sl], gt[:], xs[0:2 + LT, sl],
                                start=True, stop=True,
                            )
                        o = opool.tile([LT, W], BF16, tag="o")
                        # the tail drains split scalar/vector: by now the
                        # vector engine's carry copies are done, so the two
                        # PSUM-readers balance at ~6.7us instead of scalar
                        # serializing all 8 end drains
                        if p % 2 == 0:
                            nc.scalar.copy(out=o[:], in_=ps[0:LT, :])
                        else:
                            nc.vector.tensor_copy(out=o[:], in_=ps[0:LT, :])
                        nc.sync.dma_start(
                            out=y_d[p, L * NFULL:T, :],
                            in_=o[:],
                        )
    nc.compile()
    return nc


def _get_program():
    if "nc" not in _cache:
        _cache["nc"] = _build_program()
    return _cache["nc"]


def _ensure_axon_hooks_shim():
    """concourse's trace path does `from antenv.axon_hooks import ...`;
    some images lack that module. Install a no-op shim so an externally
    set BASS_TRACE can't crash the run (tracing then degrades to off)."""
    import types

    try:
        import antenv.axon_hooks  # noqa: F401
        return
    except ImportError:
        pass
    try:
        import antenv
    except ImportError:
        return
    mod = types.ModuleType("antenv.axon_hooks")
    mod.get_axon_ntff_profile_hook = lambda: None
    mod.set_axon_ntff_profile_hook = lambda h: None
    mod._kernel_shim = True
    sys.modules["antenv.axon_hooks"] = mod
    antenv.axon_hooks = mod


def _run(x, alpha, beta, trace=False):
    _ensure_axon_hooks_shim()
    from concourse.bass_utils import run_bass_kernel_spmd

    bf16 = _bf16()
    x = np.asarray(x, dtype=np.float32)
    # bf16-quantize, then interleave batch pairs channel-wise:
    # [64, T, C] -> [32, T, 2C] with batches (2p, 2p+1) side by side
    xb = (
        x.astype(bf16)
        .reshape(B // GB, GB, T, C)
        .transpose(0, 2, 1, 3)
        .reshape(B // GB, T, W)
    )
    G0, G1, Gt = _build_mats(alpha, beta)
    nc = _get_program()
    in_maps = [
        {"x": xb[c * NP:(c + 1) * NP], "g0": G0, "g1": G1, "gt": Gt}
        for c in range(NCORES)
    ]
    res = run_bass_kernel_spmd(nc, in_maps, list(range(NCORES)), trace=trace)
    yb = np.concatenate([res.results[c]["y"] for c in range(NCORES)], axis=0)
    out = (
        yb.reshape(B // GB, T, GB, C)
        .transpose(0, 2, 1, 3)
        .astype(np.float32)
        .reshape(B, T, C)
    )
    return out, res


def kernel(**inputs):
    alpha = float(np.asarray(inputs["alpha"]))
    beta = float(np.asarray(inputs["beta"]))
    out, _ = _run(inputs["x"], alpha, beta, trace=False)
    return out


# revision 33
# speedup vs baseline: 1.0697x; 1.0022x over previous
"""DEMA (double exponential smoothing) Trainium2 kernel.

x: [64, 2048, 512] fp32; recurrence over T=2048 is a 2x2 linear
time-invariant system per (batch, channel) lane:

    z_t = A z_{t-1} + B x_t,   y_t = e1^T z_t
    A = [[1-a, 1-a], [-ab, 1-ab]],  B = [a, ab]^T

Blocked scan: chunks of L=126 timesteps. One [128x128] @ [128x512]
matmul per (batch, chunk): rhs rows 0-1 carry the (s, b) state into
the chunk, rows 2..127 carry the chunk's inputs; lhsT columns 0-1
produce the chunk-end state (fed into the next chunk's rhs rows 0-1
via a tiny PSUM->SBUF copy), columns 2..127 produce the outputs.

The kernel is HBM-bandwidth bound (in+out traffic), so everything on
the wire is bf16: the host quantizes x and the chunk matrices to bf16
(tolerance is 2e-2 relative; bf16 end-to-end lands ~5e-3) and the
device writes bf16 outputs that the host upcasts. This halves HBM
traffic vs fp32 and runs the PE at 1 cycle/row instead of fp32's 4.
Batches are sharded 8 ways across cores; within a core the 8 batches
are packed as 4 channel-interleaved pairs so every DMA row is a 2KB
descriptor, and the 4 pair-chains are interleaved chunk-major so the
PE always has independent work while each carry chain advances.
"""

import sys

import numpy as np

if "/opt/trn_rl_repo" not in sys.path:
    sys.path.insert(0, "/opt/trn_rl_repo")

B, T, C = 64, 2048, 512
NCORES = 8
BPC = B // NCORES  # batches per core
GB = 2             # batches packed side-by-side per tile (2KB DMA rows)
NP = BPC // GB     # batch-pair chains per core
W = GB * C         # tile width in the free dim
L = 126            # timesteps per full chunk (126 outputs + 2 state rows = 128)
NFULL = 16         # full chunks cover t = 0..2015
LT = T - NFULL * L  # tail chunk, 32 timesteps

_cache = {}


def _bf16():
    from ml_dtypes import bfloat16

    return bfloat16


def _build_mats(alpha, beta):
    """Per-call host precompute of the chunk transfer matrices (float64)."""
    a = np.float64(alpha)
    b = np.float64(beta)
    A = np.array([[1 - a, 1 - a], [-a * b, 1 - a * b]], dtype=np.float64)
    Bv = np.array([a, a * b], dtype=np.float64)
    Ap = [np.eye(2)]
    for _ in range(L):
        Ap.append(Ap[-1] @ A)
    AB = np.stack([Ap[j] @ Bv for j in range(L)])  # [L, 2], A^j B
    w = AB[:, 0]                                   # w_j = e1^T A^j B

    # Generic chunk starting at t0, carry z_{t0-1} in rhs rows 0-1:
    #   z_{t0+tau} = A^{tau+1} z_{t0-1} + sum_k A^{tau-k} B x_{t0+k}
    G1 = np.zeros((128, 128))
    for tau in range(L):
        m = 2 + tau
        G1[0, m] = Ap[tau + 1][0, 0]
        G1[1, m] = Ap[tau + 1][0, 1]
        for k in range(tau + 1):
            G1[2 + k, m] = w[tau - k]
    for j in range(2):
        for jp in range(2):
            G1[j, jp] = Ap[L][jp, j]
    for k in range(L):
        G1[2 + k, 0] = AB[L - 1 - k][0]
        G1[2 + k, 1] = AB[L - 1 - k][1]

    # Chunk 0: z_0 = (x_0, x_1 - x_0), y_0 = x_0, rhs rows 0-1 are zero.
    G0 = np.zeros((128, 128))
    G0[2, 2] = 1.0
    for tau in range(1, L):
        m = 2 + tau
        G0[2, m] = Ap[tau][0, 0] - Ap[tau][0, 1]
        G0[3, m] = Ap[tau][0, 1] + w[tau - 1]
        for k in range(2, tau + 1):
            G0[2 + k, m] = w[tau - k]
    for jp in range(2):
        G0[2, jp] = Ap[L - 1][jp, 0] - Ap[L - 1][jp, 1]
        G0[3, jp] = Ap[L - 1][jp, 1] + AB[L - 2][jp]
        for k in range(2, L):
            G0[2 + k, jp] = AB[L - 1 - k][jp]

    # Tail chunk: LT outputs, no state columns.
    Gt = np.zeros((2 + LT, LT))
    for tau in range(LT):
        Gt[0, tau] = Ap[tau + 1][0, 0]
        Gt[1, tau] = Ap[tau + 1][0, 1]
        for k in range(tau + 1):
            Gt[2 + k, tau] = w[tau - k]
    bf16 = _bf16()
    # drop G0's all-zero carry rows: round 0's rhs has inputs at rows 0..L-1
    return (
        G0[2:128].astype(bf16),
        G1.astype(bf16),
        Gt.astype(bf16),
    )


def _build_program():
    import concourse.mybir as mybir
    import concourse.tile as tile
    from concourse import bacc

    FP32 = mybir.dt.float32
    BF16 = mybir.dt.bfloat16
    nc = bacc.Bacc(
        "TRN2", target_bir_lowering=False, debug=False, enable_asserts=False
    )
    x_d = nc.dram_tensor("x", [NP, T, W], BF16, kind="ExternalInput").ap()
    # g0 is the chunk-0 matrix with the (all-zero) carry rows dropped:
    # [126, 128], so round 0's rhs needs no zeroed carry rows.
    g0_d = nc.dram_tensor("g0", [L, 128], BF16, kind="ExternalInput").ap()
    g1_d = nc.dram_tensor("g1", [128, 128], BF16, kind="ExternalInput").ap()
    gt_d = nc.dram_tensor("gt", [2 + LT, LT], BF16, kind="ExternalInput").ap()
    y_d = nc.dram_tensor("y", [NP, T, W], BF16, kind="ExternalOutput").ap()

    with tile.TileContext(nc) as tc:
        with (
            tc.tile_pool(name="g", bufs=1) as gpool,
            tc.tile_pool(name="xp", bufs=12) as xpool,
            tc.tile_pool(name="op", bufs=10) as opool,
            tc.tile_pool(name="ps", bufs=4, space="PSUM") as pspool,
        ):
            g0 = gpool.tile([L, 128], BF16, tag="g0")
            g1 = gpool.tile([128, 128], BF16, tag="g1")
            gt = gpool.tile([2 + LT, LT], BF16, tag="gt")
            # issue order tuned for the startup ramp: the first matmul
            # needs only g0 + chain-0's tile, so those go first on sync;
            # scalar's queue starts behind the auto-inserted ACT table
            # load, so it gets the later-needed tiles and g1/gt
            nc.sync.dma_start(out=g0[:], in_=g0_d)

            xcur = []
            for p in range(NP):
                xs = xpool.tile([L, W], BF16, tag="x")
                eng = nc.sync if p < 2 else nc.scalar
                eng.dma_start(out=xs[:], in_=x_d[p, 0:L, :])
                xcur.append(xs)
            nc.scalar.dma_start(out=g1[:], in_=g1_d)
            nc.scalar.dma_start(out=gt[:], in_=gt_d)

            for i in range(NFULL + 1):
                for p in range(NP):
                    xs = xcur[p]
                    ps = pspool.tile([128, W], FP32, tag="ps")
                    if i < NFULL:
                        # next round's input tile for this chain; issue the
                        # prefetch DMA before this round's matmuls so the
                        # DMA queues never starve
                        if i + 1 < NFULL:
                            nxt = xpool.tile([128, W], BF16, tag="x")
                            nrows = L
                        else:
                            nxt = xpool.tile([2 + LT, W], BF16, tag="x")
                            nrows = LT
                        peng = (nc.sync if (i > 0 or p < 2) else nc.scalar)
                        peng.dma_start(
                            out=nxt[2:2 + nrows, :],
                            in_=x_d[p, L * (i + 1):L * (i + 1) + nrows, :],
                        )
                        for bb in range(GB):
                            sl = slice(bb * C, (bb + 1) * C)
                            if i == 0:
                                nc.tensor.matmul(
                                    ps[:, sl], g0[:], xs[0:L, sl],
                                    start=True, stop=True,
                                )
                            else:
                                nc.tensor.matmul(
                                    ps[:, sl], g1[:], xs[:, sl],
                                    start=True, stop=True,
                                )
                        # chunk-end states -> next rhs rows 0-1 (fp32->bf16)
                        nc.vector.tensor_copy(out=nxt[0:2, :], in_=ps[0:2, :])
                        xcur[p] = nxt
                        o = opool.tile([128, W], BF16, tag="o")
                        nc.scalar.copy(out=o[:], in_=ps[:])
                        # keep the last two rounds' outputs off gpsimd:
                        # its SWDGE end-of-kernel DRAIN takes ~5us after
                        # the final software-DGE DMA completes, while the
                        # HWDGE queues drain in ~200ns
                        # ... and keep the end output ISSUES off scalar too:
                        # its serial queue of 8 final drains plus 4 issues
                        # is exactly the measured 11.7us end-game
                        if i >= NFULL - 1:
                            eng = nc.sync
                        else:
                            eng = nc.gpsimd
                        eng.dma_start(
                            out=y_d[p, L * i:L * (i + 1), :],
                            in_=o[2:128, :],
                        )
                    else:  # tail chunk (32 steps, no state outputs)
                        for bb in range(GB):
                            sl = slice(bb * C, (bb + 1) * C)
                            nc.tensor.matmul(
                                ps[0:LT, sl], gt[:], xs[0:2 + LT, sl],
                                start=True, stop=True,
                            )
                        o = opool.tile([LT, W], BF16, tag="o")
                        # the tail drains split scalar/vector: by now the
                        # vector engine's carry copies are done, so the two
                        # PSUM-readers balance at ~6.7us instead of scalar
                        # serializing all 8 end drains
                        if p % 2 == 0:
                            nc.scalar.copy(out=o[:], in_=ps[0:LT, :])
                        else:
                            nc.vector.tensor_copy(out=o[:], in_=ps[0:LT, :])
                        nc.sync.dma_start(
                            out=y_d[p, L * NFULL:T, :],
                            in_=o[:],
                        )
    nc.compile()
    return nc


def _get_program():
    if "nc" not in _cache:
        _cache["nc"] = _build_program()
    return _cache["nc"]


def _ensure_axon_hooks_shim():
    """concourse's trace path does `from antenv.axon_hooks import ...`;
    some images lack that module. Install a no-op shim so an externally
    set BASS_TRACE can't crash the run (tracing then degrades to off)."""
    import types

    try:
        import antenv.axon_hooks  # noqa: F401
        return
    except ImportError:
        pass
    try:
        import antenv
    except ImportError:
        return
    mod = types.ModuleType("antenv.axon_hooks")
    mod.get_axon_ntff_profile_hook = lambda: None
    mod.set_axon_ntff_profile_hook = lambda h: None
    mod._kernel_shim = True
    sys.modules["antenv.axon_hooks"] = mod
    antenv.axon_hooks = mod


def _run(x, alpha, beta, trace=False):
    _ensure_axon_hooks_shim()
    from concourse.bass_utils import run_bass_kernel_spmd

    bf16 = _bf16()
    x = np.asarray(x, dtype=np.float32)
    # bf16-quantize, then interleave batch pairs channel-wise:
    # [64, T, C] -> [32, T, 2C] with batches (2p, 2p+1) side by side
    xb = (
        x.astype(bf16)
        .reshape(B // GB, GB, T, C)
        .transpose(0, 2, 1, 3)
        .reshape(B // GB, T, W)
    )
    G0, G1, Gt = _build_mats(alpha, beta)
    nc = _get_program()
    in_maps = [
        {"x": xb[c * NP:(c + 1) * NP], "g0": G0, "g1": G1, "gt": Gt}
        for c in range(NCORES)
    ]
    res = run_bass_kernel_spmd(nc, in_maps, list(range(NCORES)), trace=trace)
    yb = np.concatenate([res.results[c]["y"] for c in range(NCORES)], axis=0)
    out = (
        yb.reshape(B // GB, T, GB, C)
        .transpose(0, 2, 1, 3)
        .astype(np.float32)
        .reshape(B, T, C)
    )
    return out, res


def kernel(**inputs):
    alpha = float(np.asarray(inputs["alpha"]))
    beta = float(np.asarray(inputs["beta"]))
    out, _ = _run(inputs["x"], alpha, beta, trace=False)
    return out
